# revision 10
# baseline (speedup 1.0000x reference)
"""Trainium2 Bass kernel for ConstrainedProbabilityMatrixFactorization.

rating = uw @ iw.T + ub + ib.T + bias + (fb_values . E[fb_indices]) @ iw.T
       = ue_aug @ rhs_aug
  with ue_aug  = [uw + offset | ub + bias | 1]   [BU, 66]
       rhs_aug = [iw.T ; ones ; ib.T]            [66, BI]

Sharding: the 1024-user batch is split across 8 NeuronCores (128 users
per core). No collectives. The dominant memory work stays on device:
the per-user feedback segment-gather (50 random 256B rows per user from
item_rating_effect_weight) via two dma_gather calls (int16 index limit
-> the table is split into two 25000-row halves; out-of-half slots
gather row 0 and are zeroed via host-masked fb_values), and the 16MB
output write.

Per-core program:
  1. dma_gather x2: E rows for all 128x50 feedback slots -> gA/gB
     [128, 50, 64] (slot (p,l) at partition p, cols l*64:..).
  2. indirect gather: user_aug rows -> ue [128, 66].
  3. offset = sum_l fbvA[p,l]*gA[p,l,:] + fbvB[p,l]*gB[p,l,:]  (DVE
     broadcast multiplies + strided reduce over l).
  4. PE transpose ue -> ueT [66, 128].
  5. rhs [66, 4096]: rows 0:64 DMA'd from host-prepped iw.T batch,
     row 64 = ones, row 65 = gathered item_bias row.
  6. 8 matmuls [66,128]^T @ [66,512] -> PSUM -> SBUF -> DMA out.
"""

import numpy as np

N_USERS = 100000
N_ITEMS = 50000
HALF = N_ITEMS // 2        # 25000: dma_gather indices must fit int16
D = 64
BU = 1024
BI = 4096
L = 50
NCORES = 8
UB = BU // NCORES          # 128 users per core
P = 128
K = D + 2                  # 66: augmented contraction dim
NBANK = 8                  # output column blocks of 512
NIDX = UB * L              # 6400 feedback slots per core
NIDX16 = NIDX // 16        # 400: idx tile free dim

_cached = {}


def _build_program():
    import concourse.bacc as bacc
    import concourse.bass as bass
    import concourse.mybir as mybir
    import concourse.tile as tile
    from concourse.masks import make_identity

    f32 = mybir.dt.float32
    i32 = mybir.dt.int32
    i16 = mybir.dt.int16

    # Bacc (not raw Bass): its compile() legalizes sync waits for TRN2.
    nc = bacc.Bacc()

    uid = nc.dram_tensor("uid", [UB, 1], i32, kind="ExternalInput")
    idxa = nc.dram_tensor("idxa", [P, NIDX16], i16, kind="ExternalInput")
    idxb = nc.dram_tensor("idxb", [P, NIDX16], i16, kind="ExternalInput")
    fbva = nc.dram_tensor("fbva", [UB, L], f32, kind="ExternalInput")
    fbvb = nc.dram_tensor("fbvb", [UB, L], f32, kind="ExternalInput")
    user_aug = nc.dram_tensor("user_aug", [N_USERS, K], f32, kind="ExternalInput")
    ereA = nc.dram_tensor("ereA", [HALF, D], f32, kind="ExternalInput")
    ereB = nc.dram_tensor("ereB", [HALF, D], f32, kind="ExternalInput")
    iw_t = nc.dram_tensor("iw_t", [D, BI], f32, kind="ExternalInput")
    ib_row = nc.dram_tensor("ib_row", [1, BI], f32, kind="ExternalInput")
    rating = nc.dram_tensor("rating", [UB, BI], f32, kind="ExternalOutput")

    with tile.TileContext(nc) as tc:
        with (
            tc.tile_pool(name="sb", bufs=1) as sb,
            tc.tile_pool(name="sb_out", bufs=4) as sb_out,
            tc.tile_pool(name="ps_ue", bufs=1, space="PSUM") as ps_ue,
            tc.tile_pool(name="ps_mm", bufs=4, space="PSUM") as ps_mm,
        ):
            # --- load index / value tiles ---
            uid_s = sb.tile([P, 1], i32)
            nc.sync.dma_start(out=uid_s[:], in_=uid[:])
            ia_s = sb.tile([P, NIDX16], i16)
            nc.sync.dma_start(out=ia_s[:], in_=idxa[:])
            ib_s = sb.tile([P, NIDX16], i16)
            nc.sync.dma_start(out=ib_s[:], in_=idxb[:])
            va_s = sb.tile([P, L], f32)
            nc.sync.dma_start(out=va_s[:], in_=fbva[:])
            vb_s = sb.tile([P, L], f32)
            nc.sync.dma_start(out=vb_s[:], in_=fbvb[:])

            # --- rhs: iw.T rows + ones + ib ---
            rhs = sb.tile([K, BI], f32)
            nc.sync.dma_start(out=rhs[0:D, :], in_=iw_t[:])
            nc.vector.memset(rhs[D : D + 1, :], 1.0)
            nc.sync.dma_start(out=rhs[D + 1 : K, :], in_=ib_row[:])

            # --- feedback gathers: slot (p,l) -> partition p, cols l*64: ---
            ga = sb.tile([P, L * D], f32)
            nc.gpsimd.dma_gather(
                out_ap=ga[:].rearrange("p (l e) -> p l e", e=D),
                in_ap=ereA[:],
                idxs_ap=ia_s[:],
                num_idxs=NIDX,
                num_idxs_reg=NIDX,
                elem_size=D,
                single_packet=False,
            )
            gb = sb.tile([P, L * D], f32)
            nc.gpsimd.dma_gather(
                out_ap=gb[:].rearrange("p (l e) -> p l e", e=D),
                in_ap=ereB[:],
                idxs_ap=ib_s[:],
                num_idxs=NIDX,
                num_idxs_reg=NIDX,
                elem_size=D,
                single_packet=False,
            )

            # --- user rows: ue = [uw | ub+bias | 1] ---
            ue = sb.tile([P, K], f32)
            nc.gpsimd.indirect_dma_start(
                out=ue[:],
                out_offset=None,
                in_=user_aug[:],
                in_offset=bass.IndirectOffsetOnAxis(ap=uid_s[:], axis=0),
            )

            # --- offset[p,d] = sum_l va[p,l]*ga[p,l,d] + vb[p,l]*gb[p,l,d] ---
            prod = sb.tile([P, L * D], f32)
            nc.vector.tensor_tensor(
                out=prod[:].rearrange("p (l d) -> p l d", d=D),
                in0=ga[:].rearrange("p (l d) -> p l d", d=D),
                in1=va_s[:].to_broadcast([P, L, D]),
                op=mybir.AluOpType.mult,
            )
            prod2 = sb.tile([P, L * D], f32)
            nc.vector.tensor_tensor(
                out=prod2[:].rearrange("p (l d) -> p l d", d=D),
                in0=gb[:].rearrange("p (l d) -> p l d", d=D),
                in1=vb_s[:].to_broadcast([P, L, D]),
                op=mybir.AluOpType.mult,
            )
            nc.vector.tensor_tensor(
                out=prod[:], in0=prod[:], in1=prod2[:], op=mybir.AluOpType.add
            )
            offs = sb.tile([P, D], f32)
            nc.vector.reduce_sum(
                out=offs[:],
                in_=prod[:].rearrange("p (l d) -> p d l", d=D),
                axis=mybir.AxisListType.X,
            )
            # ue[:, :D] += offset
            nc.vector.tensor_tensor(
                out=ue[:, 0:D], in0=ue[:, 0:D], in1=offs[:],
                op=mybir.AluOpType.add,
            )

            # --- transpose ue -> ueT [66, 128] ---
            ident = sb.tile([P, P], f32)
            make_identity(nc, ident[:])
            ueT_p = ps_ue.tile([K, P], f32, space="PSUM")
            nc.tensor.transpose(out=ueT_p[:], in_=ue[:], identity=ident[:])
            ueT = sb.tile([K, P], f32)
            nc.scalar.copy(out=ueT[:], in_=ueT_p[:])

            # --- main matmuls + output ---
            for n in range(NBANK):
                mm = ps_mm.tile([P, 512], f32, space="PSUM", tag="mm")
                nc.tensor.matmul(
                    out=mm[:],
                    lhsT=ueT[:],
                    rhs=rhs[:, n * 512 : (n + 1) * 512],
                    start=True,
                    stop=True,
                )
                ot = sb_out.tile([P, 512], f32, tag="ot")
                nc.any.tensor_copy(out=ot[:], in_=mm[:])
                nc.sync.dma_start(
                    out=rating[:, n * 512 : (n + 1) * 512], in_=ot[:]
                )

    nc.finalize()
    return nc


def _get_program():
    if "nc" not in _cached:
        _cached["nc"] = _build_program()
    return _cached["nc"]


# tile[p, s] = flat[s*16 + p%16]: dma_gather index interleave, replicated
# across the 8 groups of 16 partitions.
_S_IDX = np.arange(NIDX16)[None, :] * 16 + (np.arange(P) % 16)[:, None]


def _prep_inputs(inputs):
    user_ids = np.asarray(inputs["user_ids"]).astype(np.int32)
    item_ids = np.asarray(inputs["item_ids"]).astype(np.int64)
    fb_indices = np.asarray(inputs["fb_indices"]).astype(np.int64)
    fb_values = np.asarray(inputs["fb_values"]).astype(np.float32)
    uw = np.asarray(inputs["user_weight"], dtype=np.float32)
    ub = np.asarray(inputs["user_bias"], dtype=np.float32).reshape(N_USERS, 1)
    iw = np.asarray(inputs["item_weight"], dtype=np.float32)
    ib = np.asarray(inputs["item_bias"], dtype=np.float32).reshape(N_ITEMS, 1)
    ire = np.ascontiguousarray(
        np.asarray(inputs["item_rating_effect_weight"], dtype=np.float32)
    )
    bias = float(np.asarray(inputs["bias"], dtype=np.float32).reshape(-1)[0])

    user_aug = np.empty((N_USERS, K), dtype=np.float32)
    user_aug[:, 0:D] = uw
    user_aug[:, D : D + 1] = ub + bias
    user_aug[:, D + 1] = 1.0

    # item batch: order known host-side; device streams it contiguously
    iw_t = np.ascontiguousarray(iw[item_ids].T)            # [64, 4096]
    ib_row = np.ascontiguousarray(ib[item_ids].reshape(1, BI))

    ereA = ire[:HALF]
    ereB = np.ascontiguousarray(ire[HALF:])

    in_maps = []
    for c in range(NCORES):
        sl = slice(c * UB, (c + 1) * UB)
        fbi_c = fb_indices[sl]                 # [128, 50]
        fbv_c = fb_values[sl]
        flat = fbi_c.T.reshape(-1)             # flat[l*128+p] = fbi_c[p, l]
        in_a = flat < HALF
        flatA = np.where(in_a, flat, 0).astype(np.int16)
        flatB = np.where(~in_a, flat - HALF, 0).astype(np.int16)
        mA = (fbi_c < HALF)
        in_maps.append(
            {
                "uid": user_ids[sl].reshape(UB, 1),
                "idxa": np.ascontiguousarray(flatA[_S_IDX]),
                "idxb": np.ascontiguousarray(flatB[_S_IDX]),
                "fbva": np.ascontiguousarray(fbv_c * mA),
                "fbvb": np.ascontiguousarray(fbv_c * ~mA),
                "user_aug": user_aug,
                "ereA": ereA,
                "ereB": ereB,
                "iw_t": iw_t,
                "ib_row": ib_row,
            }
        )
    return in_maps


def run(inputs, trace=False):
    """Returns (output [1024, 4096] f32, BassKernelResults)."""
    from concourse import bass_utils

    nc = _get_program()
    in_maps = _prep_inputs(inputs)
    res = bass_utils.run_bass_kernel_spmd(
        nc, in_maps, core_ids=list(range(NCORES)), trace=trace
    )
    out = np.concatenate([res.results[c]["rating"] for c in range(NCORES)], axis=0)
    return out, res


def kernel(**inputs) -> np.ndarray:
    out, _ = run(inputs, trace=False)
    return out
